# revision 43
# baseline (speedup 1.0000x reference)
"""Kernel-correlation (gnn_message_passing) Trainium2 kernel.

out[i, m] = (1/128) * sum_{l<16} exp(-||normal[i] - kernel[m, l]||^2)

Strategy: out[:, m] is a fixed smooth function of the 3-D point normal[i]
(a Gauss transform of the 1024 kernel points).  Host-side, points are
bucketed into spatial boxes (side H) and the function is expanded per box
as a total-degree-D Taylor polynomial via Hermite recurrences (fast Gauss
transform).  With D=5 there are 56 monomial features, so each box's output
is one small GEMM: out[pts, 64] = phi[pts, 56] @ C[box][56, 64].  The
device kernel is a pure TensorEngine stream -- no exp, no reduction tree:
per 256-point chunk: LDWEIGHTS(C chunk) + MATMUL -> PSUM[64, 256], then a
PSUM->SBUF bf16 copy (alternating ScalarE/VectorE) and DMA out.  Weights
are duplicated per chunk host-side so the instruction stream is uniform
and identical across the 8 SPMD cores; chunk padding makes all shapes
static.  Truncation error sits below the bf16 quantization floor
(rel err ~6e-3 vs the 2e-2 gate).

Data-parallel over chunks on 8 NeuronCores, no collectives.
"""

import time

import numpy as np

N_TOTAL = 262144
N_CORES = 8
M_KERN = 64
K_SUB = 16
MK = M_KERN * K_SUB  # 1024

H = 0.55  # box side
D = 4  # Taylor total degree
CHUNK = 256  # points per matmul chunk
N_CHUNKS0 = 160  # chunks per core (seed-0 data needs ~158); grows if overflow
BOOT_CH = 12  # chunks whose weights+features ship in the single startup DMA
OSCALE = 2040.0  # uint8 output quantization: 255 / 0.125 (theoretical max out)

TRACE = False  # set by test.py to collect a neuron profile
LAST_RESULTS = None  # BassKernelResults of the most recent run

_CACHED_NC = {}  # n_chunks -> finalized Bacc

_EXPS = np.array(
    [
        (a, b, c)
        for a in range(D + 1)
        for b in range(D + 1 - a)
        for c in range(D + 1 - a - b)
    ],
    dtype=np.int64,
)
NF = len(_EXPS)  # 35


def _build_bass(n_chunks):
    import concourse.bacc as bacc
    import concourse.mybir as mybir
    from concourse.tile import TileContext

    f32 = mybir.dt.float32
    bf16 = mybir.dt.bfloat16
    u8 = mybir.dt.uint8
    MUL = mybir.AluOpType.mult

    nc = bacc.Bacc()
    # boot carries the first BOOT_CH chunks' weights AND features so one
    # startup DMA unblocks the first matmuls (everything else competes for
    # SDMA bandwidth behind it).
    boot = nc.declare_dram_parameter(
        "boot", [NF, BOOT_CH * (M_KERN + CHUNK)], bf16, isOutput=False
    )
    phi = nc.declare_dram_parameter(
        "phi", [NF, n_chunks * CHUNK], bf16, isOutput=False
    )
    cw = nc.declare_dram_parameter(
        "cw", [NF, n_chunks * M_KERN], bf16, isOutput=False
    )
    outT = nc.declare_dram_parameter(
        "outT", [M_KERN, n_chunks * CHUNK], u8, isOutput=True
    )

    # phi arrives in pieces alternating between the two HWDGE rings so the
    # first matmuls start while the bulk is in flight; per-ring DMA
    # throughput is latency-bound (~70 GB/s), so pieces settle at 24 chunks.
    phi_pieces = [8, 8, 16, 16]
    while sum(phi_pieces) < n_chunks - BOOT_CH:
        phi_pieces.append(min(24, n_chunks - BOOT_CH - sum(phi_pieces)))

    with TileContext(nc) as tc:
        with (
            tc.tile_pool(name="inp", bufs=1) as inp,
            tc.tile_pool(name="psump", bufs=8, space="PSUM") as psump,
            tc.tile_pool(name="stagep", bufs=4) as stagep,
        ):
            boott = inp.tile([NF, BOOT_CH * (M_KERN + CHUNK)], bf16, tag="boott")
            cwt = inp.tile([NF, (n_chunks - BOOT_CH) * M_KERN], bf16, tag="cwt")
            # One tile PER phi piece: a shared tile would add tile-granular
            # write-after-read hazards that serialize piece prefetch behind
            # the matmuls consuming earlier pieces.
            # boot goes first on the sync HWDGE ring; phi pieces alternate
            # sync/scalar HWDGE rings.  Outputs ride the otherwise-idle
            # gpsimd SWDGE ring so their descriptor generation never blocks
            # input prefetch.
            nc.sync.dma_start(out=boott[:], in_=boot[:])
            # early weights ride the scalar HWDGE ring (small, lands fast,
            # doesn't delay phi piece 1 behind a bulk transfer); the rest
            # rides SWDGE ahead of the out-DMAs
            cwe = 48 * M_KERN
            nc.scalar.dma_start(
                out=cwt[:, 0:cwe],
                in_=cw[:, BOOT_CH * M_KERN : BOOT_CH * M_KERN + cwe],
            )
            phi_tiles = []  # (first_chunk, n_piece_chunks, tile)
            base = 0
            for pi, piece in enumerate(phi_pieces):
                sz = piece * CHUNK
                pt = inp.tile([NF, sz], bf16, tag=f"phi{pi}")
                src0 = BOOT_CH * CHUNK + base
                # piece 2 rides SWDGE (idle early) so the sync ring's serial
                # backlog (boot+p0) doesn't starve the pipeline ramp
                if pi == 2:
                    dq = nc.gpsimd
                else:
                    dq = nc.sync if pi % 2 == 0 else nc.scalar
                dq.dma_start(out=pt[:], in_=phi[:, src0 : src0 + sz])
                phi_tiles.append((BOOT_CH + base // CHUNK, piece, pt))
                base += sz
            # bulk weights follow the early phi on SWDGE
            nc.gpsimd.dma_start(
                out=cwt[:, cwe:], in_=cw[:, BOOT_CH * M_KERN + cwe :]
            )

            # Two 256-pt matmuls land in one [64, 512] PSUM bank; each copy
            # instruction then moves 2 chunks (halves the per-instruction
            # read-write bubble), alternating ScalarE/VectorE.  Four chunks
            # share one staging tile -> one out-DMA per 4 chunks.
            GRP = 16  # chunks per out-DMA
            assert n_chunks % GRP == 0
            ps = st = None
            piece_i = 0
            for c in range(n_chunks):
                if c < BOOT_CH:
                    lhsT = boott[:, c * M_KERN : (c + 1) * M_KERN]
                    rb = BOOT_CH * M_KERN + c * CHUNK
                    rhs = boott[:, rb : rb + CHUNK]
                else:
                    cc = c - BOOT_CH
                    lhsT = cwt[:, cc * M_KERN : (cc + 1) * M_KERN]
                    while (
                        c >= phi_tiles[piece_i][0] + phi_tiles[piece_i][1]
                    ):
                        piece_i += 1
                    pc0, _, pt = phi_tiles[piece_i]
                    off = (c - pc0) * CHUNK
                    rhs = pt[:, off : off + CHUNK]
                if c % 2 == 0:
                    ps = psump.tile([M_KERN, 2 * CHUNK], f32, tag="ps")
                nc.tensor.matmul(
                    out=ps[:, (c % 2) * CHUNK : (c % 2 + 1) * CHUNK],
                    lhsT=lhsT,
                    rhs=rhs,
                    start=True,
                    stop=True,
                )
                if c % GRP == 0:
                    st = stagep.tile([M_KERN, GRP * CHUNK], u8, tag="st")
                if c % 2 == 1:
                    sl = st[:, (c % GRP - 1) * CHUNK : (c % GRP + 1) * CHUNK]
                    # PSUM -> SBUF scaled uint8 quantized copy
                    if c % 4 == 1:
                        nc.scalar.mul(out=sl, in_=ps[:], mul=OSCALE)
                    else:
                        nc.vector.tensor_scalar(
                            out=sl, in0=ps[:], scalar1=OSCALE, scalar2=None,
                            op0=MUL,
                        )
                if c // GRP == n_chunks // GRP - 1:
                    # tail taper: the final group leaves as quarter-DMAs as
                    # soon as each 4-chunk sub-block's copies complete, on
                    # the by-then-idle HWDGE rings
                    if c % 4 == 3:
                        q0 = (c % GRP - 3) * CHUNK
                        g0c = (c - 3) * CHUNK
                        dq = nc.sync if (c % GRP) // 4 % 2 == 0 else nc.scalar
                        dq.dma_start(
                            out=outT[:, g0c : g0c + 4 * CHUNK],
                            in_=st[:, q0 : q0 + 4 * CHUNK],
                        )
                elif c % GRP == GRP - 1:
                    g0 = (c - GRP + 1) * CHUNK
                    if c >= n_chunks - 2 * GRP:
                        # second-to-last group: two half-DMAs
                        hw = GRP * CHUNK // 2
                        nc.sync.dma_start(
                            out=outT[:, g0 : g0 + hw], in_=st[:, 0:hw]
                        )
                        nc.scalar.dma_start(
                            out=outT[:, g0 + hw : g0 + 2 * hw],
                            in_=st[:, hw : 2 * hw],
                        )
                    else:
                        # ring balancing: the HWDGE rings carry the phi
                        # pieces (~1.6 MB each), so most outs ride SWDGE;
                        # a couple go to sync once its pieces thin out
                        dq = nc.sync if (c // GRP) >= 6 else nc.gpsimd
                        dq.dma_start(
                            out=outT[:, g0 : g0 + GRP * CHUNK], in_=st[:]
                        )
    return nc


def _hermite_g(t, D):
    """g_p(t) = H_p(t) e^{-t^2} / p!  for p = 0..D (physicists' Hermite)."""
    e = np.exp(-(t**2))
    H_ = np.empty((D + 1,) + t.shape)
    H_[0] = 1.0
    if D >= 1:
        H_[1] = 2 * t
    for p in range(2, D + 1):
        H_[p] = 2 * t * H_[p - 1] - 2 * (p - 1) * H_[p - 2]
    fact = np.cumprod(np.concatenate([[1.0], np.arange(1.0, D + 1)]))
    return H_ * e / fact.reshape((D + 1,) + (1,) * t.ndim)


def _prep(normal, kern):
    """Box the points, build per-box Taylor coefficients and per-point
    monomial features, lay both out as fixed-size per-chunk arrays."""
    import ml_dtypes

    bf = ml_dtypes.bfloat16
    x = np.asarray(normal, dtype=np.float64)
    kf = np.asarray(kern, dtype=np.float64).reshape(MK, 3)
    n = x.shape[0]

    L = np.abs(x).max() + 1e-6
    idx3 = np.floor((x + L) / H).astype(np.int64)
    nside = int(np.ceil(2 * L / H))
    bid = (idx3[:, 0] * nside + idx3[:, 1]) * nside + idx3[:, 2]
    uniq, inv = np.unique(bid, return_inverse=True)
    nbox = len(uniq)
    iz = uniq % nside
    iy = (uniq // nside) % nside
    ix = uniq // (nside * nside)
    centers = np.stack([ix, iy, iz], 1) * H - L + H / 2  # (nbox, 3)

    # per-box Taylor coefficients about the box center (Hermite recurrence),
    # summed over each m's 16 kernel points; includes the 1/128 out-scale
    t = kf[None, :, :] - centers[:, None, :]  # (nbox, 1024, 3)
    g = _hermite_g(t, D)  # (D+1, nbox, 1024, 3)
    prod = g[_EXPS[:, 0], :, :, 0] * g[_EXPS[:, 1], :, :, 1] * g[_EXPS[:, 2], :, :, 2]
    C = np.transpose(
        prod.reshape(NF, nbox, M_KERN, K_SUB).sum(-1), (1, 0, 2)
    )  # (nbox, NF, 64)
    C = np.ascontiguousarray(C / 128.0)

    # per-point monomial features of (x - center(box))
    delta = x - centers[inv]
    powd = [np.vander(delta[:, d], D + 1, increasing=True) for d in range(3)]
    feats = (
        powd[0][:, _EXPS[:, 0]] * powd[1][:, _EXPS[:, 1]] * powd[2][:, _EXPS[:, 2]]
    )  # (n, NF)

    # chunk layout: points sorted by box, each box padded to CHUNK multiple
    order = np.argsort(inv, kind="stable")
    cnt = np.bincount(inv, minlength=nbox)
    box_chunks = -(-cnt // CHUNK)  # ceil
    total_chunks = int(box_chunks.sum())
    n_chunks = N_CHUNKS0
    while n_chunks * N_CORES < total_chunks:
        n_chunks += 32
    cap = n_chunks * N_CORES

    chunk_box = np.full(cap, -1, dtype=np.int64)
    slot_pid = np.full(cap * CHUNK, -1, dtype=np.int64)
    chunk_starts = np.concatenate([[0], np.cumsum(box_chunks)])[:-1]
    pt_starts = np.concatenate([[0], np.cumsum(cnt)])[:-1]
    # vectorized scatter of point-ids into padded slots
    seq = np.arange(n)
    box_of_pt = inv[order]
    rank_in_box = seq - pt_starts[box_of_pt]
    slot = (
        chunk_starts[box_of_pt] * CHUNK
        + (rank_in_box // CHUNK) * CHUNK
        + rank_in_box % CHUNK
    )
    slot_pid[slot] = order
    for b_ids, c_starts, c_counts in [(np.arange(nbox), chunk_starts, box_chunks)]:
        reps = np.repeat(b_ids, c_counts)
        chunk_box[: len(reps)] = reps

    # phi: (NF, cap*CHUNK) bf16, zero on padding
    phi = np.zeros((NF, cap * CHUNK), dtype=bf)
    valid = slot_pid >= 0
    phi[:, valid] = feats[slot_pid[valid]].astype(bf).T

    # cw: (NF, cap*64) bf16, per-chunk duplicated box coefficients
    cw = np.zeros((NF, cap, M_KERN), dtype=bf)
    vc = chunk_box >= 0
    cw[:, vc, :] = C[chunk_box[vc]].astype(bf).transpose(1, 0, 2)
    cw = cw.reshape(NF, cap * M_KERN)

    return phi, cw, slot_pid, n_chunks


def kernel(normal, neighbour, kernel):  # noqa: A002 - harness-fixed names
    global LAST_RESULTS
    from concourse.bass_utils import run_bass_kernel_spmd

    n = np.asarray(normal).shape[0]
    phi, cw, slot_pid, n_chunks = _prep(normal, kernel)

    if n_chunks not in _CACHED_NC:
        ncb = _build_bass(n_chunks)
        if not ncb.is_finalized():
            ncb.finalize()
        _CACHED_NC[n_chunks] = ncb
    ncb = _CACHED_NC[n_chunks]

    cs = n_chunks * CHUNK
    ws = n_chunks * M_KERN
    bw = BOOT_CH * M_KERN
    bc = BOOT_CH * CHUNK
    in_maps = []
    for i in range(N_CORES):
        phi_i = phi[:, i * cs : (i + 1) * cs]
        cw_i = cw[:, i * ws : (i + 1) * ws]
        in_maps.append(
            {
                "boot": np.ascontiguousarray(
                    np.concatenate([cw_i[:, :bw], phi_i[:, :bc]], axis=1)
                ),
                "phi": np.ascontiguousarray(phi_i),
                "cw": np.ascontiguousarray(cw_i),
            }
        )
    # The device occasionally throws a transient NRT_EXEC_UNIT_UNRECOVERABLE;
    # observed to clear after a short wait, so retry rather than fail.
    last_exc = None
    for attempt in range(3):
        if attempt:
            time.sleep(20)
        try:
            res = run_bass_kernel_spmd(
                ncb, in_maps, list(range(N_CORES)), trace=TRACE
            )
            break
        except (ImportError, TypeError, ValueError, AssertionError):
            raise
        except Exception as e:  # noqa: BLE001 - transient runtime faults
            last_exc = e
    else:
        raise last_exc
    LAST_RESULTS = res

    outT = np.concatenate(
        [res.results[i]["outT"] for i in range(N_CORES)], axis=1
    )  # (64, cap*CHUNK) uint8
    out = np.empty((n, M_KERN), dtype=np.float32)
    valid = slot_pid >= 0
    out[slot_pid[valid]] = outT[:, valid].T.astype(np.float32) * (1.0 / OSCALE)
    return np.ascontiguousarray(out)


# revision 47
# speedup vs baseline: 1.0126x; 1.0126x over previous
"""Kernel-correlation (gnn_message_passing) Trainium2 kernel.

out[i, m] = (1/128) * sum_{l<16} exp(-||normal[i] - kernel[m, l]||^2)

Strategy: out[:, m] is a fixed smooth function of the 3-D point normal[i]
(a Gauss transform of the 1024 kernel points).  Host-side, points are
bucketed into spatial boxes (side H) and the function is expanded per box
as a total-degree-D Taylor polynomial via Hermite recurrences (fast Gauss
transform).  With D=4 there are 35 monomial features, so each box's output
is one small GEMM: out[pts, 64] = phi[pts, 35] @ C[box][35, 64].  The
device kernel is a pure TensorEngine stream -- no exp, no reduction tree:
per 256-point chunk: LDWEIGHTS(C chunk) + MATMUL -> PSUM[64, 256], then a
PSUM->SBUF scaled-uint8 copy (alternating ScalarE/VectorE) and DMA out
(host dequantizes).  Weights are duplicated per chunk host-side so the
instruction stream is uniform and identical across the 8 SPMD cores; chunk
padding makes all shapes static.  DMA ring scheduling (boot DMA, piece
sizing, three-ring balancing) hides input latency behind the matmul
stream.  Total error ~1.0e-2 rel vs the 2e-2 gate.

Data-parallel over chunks on 8 NeuronCores, no collectives.
"""

import time

import numpy as np

N_TOTAL = 262144
N_CORES = 8
M_KERN = 64
K_SUB = 16
MK = M_KERN * K_SUB  # 1024

H = 0.55  # box side
D = 4  # Taylor total degree
CHUNK = 256  # points per matmul chunk
N_CHUNKS0 = 160  # chunks per core (seed-0 data needs ~158); grows if overflow
BOOT_CH = 12  # chunks whose weights+features ship in the single startup DMA
OSCALE = 2040.0  # uint8 output quantization: 255 / 0.125 (theoretical max out)

TRACE = False  # set by test.py to collect a neuron profile
LAST_RESULTS = None  # BassKernelResults of the most recent run

_CACHED_NC = {}  # n_chunks -> finalized Bacc

_EXPS = np.array(
    [
        (a, b, c)
        for a in range(D + 1)
        for b in range(D + 1 - a)
        for c in range(D + 1 - a - b)
    ],
    dtype=np.int64,
)
NF = len(_EXPS)  # 35


def _build_bass(n_chunks):
    import concourse.bacc as bacc
    import concourse.mybir as mybir
    from concourse.tile import TileContext

    f32 = mybir.dt.float32
    bf16 = mybir.dt.bfloat16
    u8 = mybir.dt.uint8
    MUL = mybir.AluOpType.mult

    nc = bacc.Bacc()
    # boot carries the first BOOT_CH chunks' weights AND features so one
    # startup DMA unblocks the first matmuls (everything else competes for
    # SDMA bandwidth behind it).
    boot = nc.declare_dram_parameter(
        "boot", [NF, BOOT_CH * (M_KERN + CHUNK)], bf16, isOutput=False
    )
    phi = nc.declare_dram_parameter(
        "phi", [NF, n_chunks * CHUNK], bf16, isOutput=False
    )
    cw = nc.declare_dram_parameter(
        "cw", [NF, n_chunks * M_KERN], bf16, isOutput=False
    )
    outT = nc.declare_dram_parameter(
        "outT", [M_KERN, n_chunks * CHUNK], u8, isOutput=True
    )

    # phi arrives in pieces alternating between the two HWDGE rings so the
    # first matmuls start while the bulk is in flight; per-ring DMA
    # throughput is latency-bound (~70 GB/s), so pieces settle at 24 chunks.
    phi_pieces = [8, 8, 16, 16]
    while sum(phi_pieces) < n_chunks - BOOT_CH:
        phi_pieces.append(min(24, n_chunks - BOOT_CH - sum(phi_pieces)))

    with TileContext(nc) as tc:
        with (
            tc.tile_pool(name="inp", bufs=1) as inp,
            tc.tile_pool(name="psump", bufs=8, space="PSUM") as psump,
            tc.tile_pool(name="stagep", bufs=4) as stagep,
        ):
            boott = inp.tile([NF, BOOT_CH * (M_KERN + CHUNK)], bf16, tag="boott")
            cwt = inp.tile([NF, (n_chunks - BOOT_CH) * M_KERN], bf16, tag="cwt")
            # One tile PER phi piece: a shared tile would add tile-granular
            # write-after-read hazards that serialize piece prefetch behind
            # the matmuls consuming earlier pieces.
            # boot goes first on the sync HWDGE ring; phi pieces alternate
            # sync/scalar HWDGE rings.  Outputs mostly ride the gpsimd SWDGE
            # ring so their descriptor generation never blocks input
            # prefetch.
            nc.sync.dma_start(out=boott[:], in_=boot[:])
            # early weights ride the scalar HWDGE ring (small, lands fast,
            # doesn't delay phi piece 1 behind a bulk transfer); the rest
            # rides SWDGE ahead of the out-DMAs
            cwe = 32 * M_KERN
            nc.scalar.dma_start(
                out=cwt[:, 0:cwe],
                in_=cw[:, BOOT_CH * M_KERN : BOOT_CH * M_KERN + cwe],
            )
            nc.gpsimd.dma_start(
                out=cwt[:, cwe:], in_=cw[:, BOOT_CH * M_KERN + cwe :]
            )
            phi_tiles = []  # (first_chunk, n_piece_chunks, tile)
            base = 0
            for pi, piece in enumerate(phi_pieces):
                sz = piece * CHUNK
                pt = inp.tile([NF, sz], bf16, tag=f"phi{pi}")
                src0 = BOOT_CH * CHUNK + base
                dq = nc.sync if pi % 2 == 0 else nc.scalar
                dq.dma_start(out=pt[:], in_=phi[:, src0 : src0 + sz])
                phi_tiles.append((BOOT_CH + base // CHUNK, piece, pt))
                base += sz

            # Two 256-pt matmuls land in one [64, 512] PSUM bank; each copy
            # instruction then moves 2 chunks (halves the per-instruction
            # read-write bubble), alternating ScalarE/VectorE.  GRP chunks
            # share one staging tile -> one out-DMA per GRP chunks.
            GRP = 16  # chunks per out-DMA
            assert n_chunks % GRP == 0
            ps = st = None
            piece_i = 0
            for c in range(n_chunks):
                if c < BOOT_CH:
                    lhsT = boott[:, c * M_KERN : (c + 1) * M_KERN]
                    rb = BOOT_CH * M_KERN + c * CHUNK
                    rhs = boott[:, rb : rb + CHUNK]
                else:
                    cc = c - BOOT_CH
                    lhsT = cwt[:, cc * M_KERN : (cc + 1) * M_KERN]
                    while (
                        c >= phi_tiles[piece_i][0] + phi_tiles[piece_i][1]
                    ):
                        piece_i += 1
                    pc0, _, pt = phi_tiles[piece_i]
                    off = (c - pc0) * CHUNK
                    rhs = pt[:, off : off + CHUNK]
                if c % 2 == 0:
                    ps = psump.tile([M_KERN, 2 * CHUNK], f32, tag="ps")
                nc.tensor.matmul(
                    out=ps[:, (c % 2) * CHUNK : (c % 2 + 1) * CHUNK],
                    lhsT=lhsT,
                    rhs=rhs,
                    start=True,
                    stop=True,
                )
                if c % GRP == 0:
                    st = stagep.tile([M_KERN, GRP * CHUNK], u8, tag="st")
                if c % 2 == 1:
                    sl = st[:, (c % GRP - 1) * CHUNK : (c % GRP + 1) * CHUNK]
                    # PSUM -> SBUF scaled uint8 quantized copy
                    if c % 4 == 1:
                        nc.scalar.mul(out=sl, in_=ps[:], mul=OSCALE)
                    else:
                        nc.vector.tensor_scalar(
                            out=sl, in0=ps[:], scalar1=OSCALE, scalar2=None,
                            op0=MUL,
                        )
                if c % GRP == GRP - 1:
                    g0 = (c - GRP + 1) * CHUNK
                    if c >= n_chunks - 2 * GRP:
                        # tail taper: final two groups leave as half-DMAs
                        # on the by-then-idle HWDGE rings
                        hw = GRP * CHUNK // 2
                        nc.sync.dma_start(
                            out=outT[:, g0 : g0 + hw], in_=st[:, 0:hw]
                        )
                        nc.scalar.dma_start(
                            out=outT[:, g0 + hw : g0 + 2 * hw],
                            in_=st[:, hw : 2 * hw],
                        )
                    else:
                        # ring balancing: the HWDGE rings carry the phi
                        # pieces (~1.4 MB each), so most outs ride SWDGE;
                        # a couple go to sync once its pieces thin out
                        dq = nc.sync if (c // GRP) >= 6 else nc.gpsimd
                        dq.dma_start(
                            out=outT[:, g0 : g0 + GRP * CHUNK], in_=st[:]
                        )
    return nc


def _hermite_g(t, D):
    """g_p(t) = H_p(t) e^{-t^2} / p!  for p = 0..D (physicists' Hermite)."""
    e = np.exp(-(t**2))
    H_ = np.empty((D + 1,) + t.shape)
    H_[0] = 1.0
    if D >= 1:
        H_[1] = 2 * t
    for p in range(2, D + 1):
        H_[p] = 2 * t * H_[p - 1] - 2 * (p - 1) * H_[p - 2]
    fact = np.cumprod(np.concatenate([[1.0], np.arange(1.0, D + 1)]))
    return H_ * e / fact.reshape((D + 1,) + (1,) * t.ndim)


def _prep(normal, kern):
    """Box the points, build per-box Taylor coefficients and per-point
    monomial features, lay both out as fixed-size per-chunk arrays."""
    import ml_dtypes

    bf = ml_dtypes.bfloat16
    x = np.asarray(normal, dtype=np.float64)
    kf = np.asarray(kern, dtype=np.float64).reshape(MK, 3)
    n = x.shape[0]

    L = np.abs(x).max() + 1e-6
    idx3 = np.floor((x + L) / H).astype(np.int64)
    nside = int(np.ceil(2 * L / H))
    bid = (idx3[:, 0] * nside + idx3[:, 1]) * nside + idx3[:, 2]
    uniq, inv = np.unique(bid, return_inverse=True)
    nbox = len(uniq)
    iz = uniq % nside
    iy = (uniq // nside) % nside
    ix = uniq // (nside * nside)
    centers = np.stack([ix, iy, iz], 1) * H - L + H / 2  # (nbox, 3)

    # per-box Taylor coefficients about the box center (Hermite recurrence),
    # summed over each m's 16 kernel points; includes the 1/128 out-scale
    t = kf[None, :, :] - centers[:, None, :]  # (nbox, 1024, 3)
    g = _hermite_g(t, D)  # (D+1, nbox, 1024, 3)
    prod = g[_EXPS[:, 0], :, :, 0] * g[_EXPS[:, 1], :, :, 1] * g[_EXPS[:, 2], :, :, 2]
    C = np.transpose(
        prod.reshape(NF, nbox, M_KERN, K_SUB).sum(-1), (1, 0, 2)
    )  # (nbox, NF, 64)
    C = np.ascontiguousarray(C / 128.0)

    # per-point monomial features of (x - center(box))
    delta = x - centers[inv]
    powd = [np.vander(delta[:, d], D + 1, increasing=True) for d in range(3)]
    feats = (
        powd[0][:, _EXPS[:, 0]] * powd[1][:, _EXPS[:, 1]] * powd[2][:, _EXPS[:, 2]]
    )  # (n, NF)

    # chunk layout: points sorted by box, each box padded to CHUNK multiple
    order = np.argsort(inv, kind="stable")
    cnt = np.bincount(inv, minlength=nbox)
    box_chunks = -(-cnt // CHUNK)  # ceil
    total_chunks = int(box_chunks.sum())
    n_chunks = N_CHUNKS0
    while n_chunks * N_CORES < total_chunks:
        n_chunks += 32
    cap = n_chunks * N_CORES

    chunk_box = np.full(cap, -1, dtype=np.int64)
    slot_pid = np.full(cap * CHUNK, -1, dtype=np.int64)
    chunk_starts = np.concatenate([[0], np.cumsum(box_chunks)])[:-1]
    pt_starts = np.concatenate([[0], np.cumsum(cnt)])[:-1]
    # vectorized scatter of point-ids into padded slots
    seq = np.arange(n)
    box_of_pt = inv[order]
    rank_in_box = seq - pt_starts[box_of_pt]
    slot = (
        chunk_starts[box_of_pt] * CHUNK
        + (rank_in_box // CHUNK) * CHUNK
        + rank_in_box % CHUNK
    )
    slot_pid[slot] = order
    for b_ids, c_starts, c_counts in [(np.arange(nbox), chunk_starts, box_chunks)]:
        reps = np.repeat(b_ids, c_counts)
        chunk_box[: len(reps)] = reps

    # phi: (NF, cap*CHUNK) bf16, zero on padding
    phi = np.zeros((NF, cap * CHUNK), dtype=bf)
    valid = slot_pid >= 0
    phi[:, valid] = feats[slot_pid[valid]].astype(bf).T

    # cw: (NF, cap*64) bf16, per-chunk duplicated box coefficients
    cw = np.zeros((NF, cap, M_KERN), dtype=bf)
    vc = chunk_box >= 0
    cw[:, vc, :] = C[chunk_box[vc]].astype(bf).transpose(1, 0, 2)
    cw = cw.reshape(NF, cap * M_KERN)

    return phi, cw, slot_pid, n_chunks


def kernel(normal, neighbour, kernel):  # noqa: A002 - harness-fixed names
    global LAST_RESULTS
    from concourse.bass_utils import run_bass_kernel_spmd

    n = np.asarray(normal).shape[0]
    phi, cw, slot_pid, n_chunks = _prep(normal, kernel)

    if n_chunks not in _CACHED_NC:
        ncb = _build_bass(n_chunks)
        if not ncb.is_finalized():
            ncb.finalize()
        _CACHED_NC[n_chunks] = ncb
    ncb = _CACHED_NC[n_chunks]

    cs = n_chunks * CHUNK
    ws = n_chunks * M_KERN
    bw = BOOT_CH * M_KERN
    bc = BOOT_CH * CHUNK
    in_maps = []
    for i in range(N_CORES):
        phi_i = phi[:, i * cs : (i + 1) * cs]
        cw_i = cw[:, i * ws : (i + 1) * ws]
        in_maps.append(
            {
                "boot": np.ascontiguousarray(
                    np.concatenate([cw_i[:, :bw], phi_i[:, :bc]], axis=1)
                ),
                "phi": np.ascontiguousarray(phi_i),
                "cw": np.ascontiguousarray(cw_i),
            }
        )
    # The device occasionally throws a transient NRT_EXEC_UNIT_UNRECOVERABLE;
    # observed to clear after a short wait, so retry rather than fail.
    last_exc = None
    for attempt in range(3):
        if attempt:
            time.sleep(20)
        try:
            res = run_bass_kernel_spmd(
                ncb, in_maps, list(range(N_CORES)), trace=TRACE
            )
            break
        except (ImportError, TypeError, ValueError, AssertionError):
            raise
        except Exception as e:  # noqa: BLE001 - transient runtime faults
            last_exc = e
    else:
        raise last_exc
    LAST_RESULTS = res

    outT = np.concatenate(
        [res.results[i]["outT"] for i in range(N_CORES)], axis=1
    )  # (64, cap*CHUNK) uint8
    out = np.empty((n, M_KERN), dtype=np.float32)
    valid = slot_pid >= 0
    out[slot_pid[valid]] = outT[:, valid].T.astype(np.float32) * (1.0 / OSCALE)
    return np.ascontiguousarray(out)


# revision 48
# speedup vs baseline: 1.0508x; 1.0377x over previous
"""Kernel-correlation (gnn_message_passing) Trainium2 kernel.

out[i, m] = (1/128) * sum_{l<16} exp(-||normal[i] - kernel[m, l]||^2)

Strategy: out[:, m] is a fixed smooth function of the 3-D point normal[i]
(a Gauss transform of the 1024 kernel points).  Host-side, points are
bucketed into spatial boxes (side H) and the function is expanded per box
as a total-degree-D Taylor polynomial via Hermite recurrences (fast Gauss
transform).  With D=4 there are 35 monomial features, so each box's output
is one small GEMM: out[pts, 64] = phi[pts, 35] @ C[box][35, 64].  The
device kernel is a pure TensorEngine stream -- no exp, no reduction tree:
per 256-point chunk: LDWEIGHTS(C chunk) + MATMUL -> PSUM[64, 256], then a
PSUM->SBUF scaled-uint8 copy (alternating ScalarE/VectorE) and DMA out
(host dequantizes).  Weights are duplicated per chunk host-side so the
instruction stream is uniform and identical across the 8 SPMD cores; chunk
padding makes all shapes static.  DMA ring scheduling (boot DMA, piece
sizing, three-ring balancing) hides input latency behind the matmul
stream.  Total error ~1.0e-2 rel vs the 2e-2 gate.

Data-parallel over chunks on 8 NeuronCores, no collectives.
"""

import time

import numpy as np

N_TOTAL = 262144
N_CORES = 8
M_KERN = 64
K_SUB = 16
MK = M_KERN * K_SUB  # 1024

H = 0.55  # box side
D = 4  # Taylor total degree
CHUNK = 256  # points per matmul chunk
N_CHUNKS0 = 160  # chunks per core (seed-0 data needs ~158); grows if overflow
BOOT_CH = 12  # chunks whose weights+features ship in the single startup DMA
OSCALE = 2040.0  # uint8 output quantization: 255 / 0.125 (theoretical max out)

TRACE = False  # set by test.py to collect a neuron profile
LAST_RESULTS = None  # BassKernelResults of the most recent run

_CACHED_NC = {}  # n_chunks -> finalized Bacc

_EXPS = np.array(
    [
        (a, b, c)
        for a in range(D + 1)
        for b in range(D + 1 - a)
        for c in range(D + 1 - a - b)
    ],
    dtype=np.int64,
)
NF = len(_EXPS)  # 35


def _build_bass(n_chunks):
    import concourse.bacc as bacc
    import concourse.mybir as mybir
    from concourse.tile import TileContext

    f32 = mybir.dt.float32
    bf16 = mybir.dt.bfloat16
    u8 = mybir.dt.uint8
    MUL = mybir.AluOpType.mult

    nc = bacc.Bacc()
    # boot carries the first BOOT_CH chunks' weights AND features so one
    # startup DMA unblocks the first matmuls (everything else competes for
    # SDMA bandwidth behind it).
    boot = nc.declare_dram_parameter(
        "boot", [NF, BOOT_CH * (M_KERN + CHUNK)], bf16, isOutput=False
    )
    phi = nc.declare_dram_parameter(
        "phi", [NF, n_chunks * CHUNK], bf16, isOutput=False
    )
    cw = nc.declare_dram_parameter(
        "cw", [NF, n_chunks * M_KERN], bf16, isOutput=False
    )
    outT = nc.declare_dram_parameter(
        "outT", [M_KERN, n_chunks * CHUNK], u8, isOutput=True
    )

    # phi arrives in pieces alternating between the two HWDGE rings so the
    # first matmuls start while the bulk is in flight.  Sizes are chosen so
    # each ring's cumulative serial delivery (~70 GB/s/ring) stays ahead of
    # the matmul stream's consumption curve: small pieces early, growing.
    phi_pieces = [8, 8, 8, 16, 16]
    while sum(phi_pieces) < n_chunks - BOOT_CH:
        phi_pieces.append(min(24, n_chunks - BOOT_CH - sum(phi_pieces)))

    with TileContext(nc) as tc:
        with (
            tc.tile_pool(name="inp", bufs=1) as inp,
            tc.tile_pool(name="psump", bufs=8, space="PSUM") as psump,
            tc.tile_pool(name="stagep", bufs=4) as stagep,
        ):
            boott = inp.tile([NF, BOOT_CH * (M_KERN + CHUNK)], bf16, tag="boott")
            cwt = inp.tile([NF, (n_chunks - BOOT_CH) * M_KERN], bf16, tag="cwt")
            # One tile PER phi piece: a shared tile would add tile-granular
            # write-after-read hazards that serialize piece prefetch behind
            # the matmuls consuming earlier pieces.
            # boot goes first on the sync HWDGE ring; phi pieces alternate
            # sync/scalar HWDGE rings.  Outputs mostly ride the gpsimd SWDGE
            # ring so their descriptor generation never blocks input
            # prefetch.
            nc.sync.dma_start(out=boott[:], in_=boot[:])
            # early weights ride the scalar HWDGE ring (small, lands fast,
            # doesn't delay phi piece 1 behind a bulk transfer); the rest
            # rides SWDGE ahead of the out-DMAs
            cwe = 32 * M_KERN
            nc.scalar.dma_start(
                out=cwt[:, 0:cwe],
                in_=cw[:, BOOT_CH * M_KERN : BOOT_CH * M_KERN + cwe],
            )
            nc.gpsimd.dma_start(
                out=cwt[:, cwe:], in_=cw[:, BOOT_CH * M_KERN + cwe :]
            )
            phi_tiles = []  # (first_chunk, n_piece_chunks, tile)
            base = 0
            for pi, piece in enumerate(phi_pieces):
                sz = piece * CHUNK
                pt = inp.tile([NF, sz], bf16, tag=f"phi{pi}")
                src0 = BOOT_CH * CHUNK + base
                dq = nc.sync if pi % 2 == 0 else nc.scalar
                dq.dma_start(out=pt[:], in_=phi[:, src0 : src0 + sz])
                phi_tiles.append((BOOT_CH + base // CHUNK, piece, pt))
                base += sz

            # Two 256-pt matmuls land in one [64, 512] PSUM bank; each copy
            # instruction then moves 2 chunks (halves the per-instruction
            # read-write bubble), alternating ScalarE/VectorE.  GRP chunks
            # share one staging tile -> one out-DMA per GRP chunks.
            GRP = 16  # chunks per out-DMA
            assert n_chunks % GRP == 0
            ps = st = None
            piece_i = 0
            for c in range(n_chunks):
                if c < BOOT_CH:
                    lhsT = boott[:, c * M_KERN : (c + 1) * M_KERN]
                    rb = BOOT_CH * M_KERN + c * CHUNK
                    rhs = boott[:, rb : rb + CHUNK]
                else:
                    cc = c - BOOT_CH
                    lhsT = cwt[:, cc * M_KERN : (cc + 1) * M_KERN]
                    while (
                        c >= phi_tiles[piece_i][0] + phi_tiles[piece_i][1]
                    ):
                        piece_i += 1
                    pc0, _, pt = phi_tiles[piece_i]
                    off = (c - pc0) * CHUNK
                    rhs = pt[:, off : off + CHUNK]
                if c % 2 == 0:
                    ps = psump.tile([M_KERN, 2 * CHUNK], f32, tag="ps")
                nc.tensor.matmul(
                    out=ps[:, (c % 2) * CHUNK : (c % 2 + 1) * CHUNK],
                    lhsT=lhsT,
                    rhs=rhs,
                    start=True,
                    stop=True,
                )
                if c % GRP == 0:
                    st = stagep.tile([M_KERN, GRP * CHUNK], u8, tag="st")
                if c % 2 == 1:
                    sl = st[:, (c % GRP - 1) * CHUNK : (c % GRP + 1) * CHUNK]
                    # PSUM -> SBUF scaled uint8 quantized copy
                    if c % 4 == 1:
                        nc.scalar.mul(out=sl, in_=ps[:], mul=OSCALE)
                    else:
                        nc.vector.tensor_scalar(
                            out=sl, in0=ps[:], scalar1=OSCALE, scalar2=None,
                            op0=MUL,
                        )
                if c % GRP == GRP - 1:
                    g0 = (c - GRP + 1) * CHUNK
                    if c >= n_chunks - 2 * GRP:
                        # tail taper: final two groups leave as half-DMAs
                        # on the by-then-idle HWDGE rings
                        hw = GRP * CHUNK // 2
                        nc.sync.dma_start(
                            out=outT[:, g0 : g0 + hw], in_=st[:, 0:hw]
                        )
                        nc.scalar.dma_start(
                            out=outT[:, g0 + hw : g0 + 2 * hw],
                            in_=st[:, hw : 2 * hw],
                        )
                    else:
                        # ring balancing: the HWDGE rings carry the phi
                        # pieces (~1.4 MB each), so most outs ride SWDGE;
                        # a couple go to sync once its pieces thin out
                        dq = nc.sync if (c // GRP) >= 6 else nc.gpsimd
                        dq.dma_start(
                            out=outT[:, g0 : g0 + GRP * CHUNK], in_=st[:]
                        )
    return nc


def _hermite_g(t, D):
    """g_p(t) = H_p(t) e^{-t^2} / p!  for p = 0..D (physicists' Hermite)."""
    e = np.exp(-(t**2))
    H_ = np.empty((D + 1,) + t.shape)
    H_[0] = 1.0
    if D >= 1:
        H_[1] = 2 * t
    for p in range(2, D + 1):
        H_[p] = 2 * t * H_[p - 1] - 2 * (p - 1) * H_[p - 2]
    fact = np.cumprod(np.concatenate([[1.0], np.arange(1.0, D + 1)]))
    return H_ * e / fact.reshape((D + 1,) + (1,) * t.ndim)


def _prep(normal, kern):
    """Box the points, build per-box Taylor coefficients and per-point
    monomial features, lay both out as fixed-size per-chunk arrays."""
    import ml_dtypes

    bf = ml_dtypes.bfloat16
    x = np.asarray(normal, dtype=np.float64)
    kf = np.asarray(kern, dtype=np.float64).reshape(MK, 3)
    n = x.shape[0]

    L = np.abs(x).max() + 1e-6
    idx3 = np.floor((x + L) / H).astype(np.int64)
    nside = int(np.ceil(2 * L / H))
    bid = (idx3[:, 0] * nside + idx3[:, 1]) * nside + idx3[:, 2]
    uniq, inv = np.unique(bid, return_inverse=True)
    nbox = len(uniq)
    iz = uniq % nside
    iy = (uniq // nside) % nside
    ix = uniq // (nside * nside)
    centers = np.stack([ix, iy, iz], 1) * H - L + H / 2  # (nbox, 3)

    # per-box Taylor coefficients about the box center (Hermite recurrence),
    # summed over each m's 16 kernel points; includes the 1/128 out-scale
    t = kf[None, :, :] - centers[:, None, :]  # (nbox, 1024, 3)
    g = _hermite_g(t, D)  # (D+1, nbox, 1024, 3)
    prod = g[_EXPS[:, 0], :, :, 0] * g[_EXPS[:, 1], :, :, 1] * g[_EXPS[:, 2], :, :, 2]
    C = np.transpose(
        prod.reshape(NF, nbox, M_KERN, K_SUB).sum(-1), (1, 0, 2)
    )  # (nbox, NF, 64)
    C = np.ascontiguousarray(C / 128.0)

    # per-point monomial features of (x - center(box))
    delta = x - centers[inv]
    powd = [np.vander(delta[:, d], D + 1, increasing=True) for d in range(3)]
    feats = (
        powd[0][:, _EXPS[:, 0]] * powd[1][:, _EXPS[:, 1]] * powd[2][:, _EXPS[:, 2]]
    )  # (n, NF)

    # chunk layout: points sorted by box, each box padded to CHUNK multiple
    order = np.argsort(inv, kind="stable")
    cnt = np.bincount(inv, minlength=nbox)
    box_chunks = -(-cnt // CHUNK)  # ceil
    total_chunks = int(box_chunks.sum())
    n_chunks = N_CHUNKS0
    while n_chunks * N_CORES < total_chunks:
        n_chunks += 32
    cap = n_chunks * N_CORES

    chunk_box = np.full(cap, -1, dtype=np.int64)
    slot_pid = np.full(cap * CHUNK, -1, dtype=np.int64)
    chunk_starts = np.concatenate([[0], np.cumsum(box_chunks)])[:-1]
    pt_starts = np.concatenate([[0], np.cumsum(cnt)])[:-1]
    # vectorized scatter of point-ids into padded slots
    seq = np.arange(n)
    box_of_pt = inv[order]
    rank_in_box = seq - pt_starts[box_of_pt]
    slot = (
        chunk_starts[box_of_pt] * CHUNK
        + (rank_in_box // CHUNK) * CHUNK
        + rank_in_box % CHUNK
    )
    slot_pid[slot] = order
    for b_ids, c_starts, c_counts in [(np.arange(nbox), chunk_starts, box_chunks)]:
        reps = np.repeat(b_ids, c_counts)
        chunk_box[: len(reps)] = reps

    # phi: (NF, cap*CHUNK) bf16, zero on padding
    phi = np.zeros((NF, cap * CHUNK), dtype=bf)
    valid = slot_pid >= 0
    phi[:, valid] = feats[slot_pid[valid]].astype(bf).T

    # cw: (NF, cap*64) bf16, per-chunk duplicated box coefficients
    cw = np.zeros((NF, cap, M_KERN), dtype=bf)
    vc = chunk_box >= 0
    cw[:, vc, :] = C[chunk_box[vc]].astype(bf).transpose(1, 0, 2)
    cw = cw.reshape(NF, cap * M_KERN)

    return phi, cw, slot_pid, n_chunks


def kernel(normal, neighbour, kernel):  # noqa: A002 - harness-fixed names
    global LAST_RESULTS
    from concourse.bass_utils import run_bass_kernel_spmd

    n = np.asarray(normal).shape[0]
    phi, cw, slot_pid, n_chunks = _prep(normal, kernel)

    if n_chunks not in _CACHED_NC:
        ncb = _build_bass(n_chunks)
        if not ncb.is_finalized():
            ncb.finalize()
        _CACHED_NC[n_chunks] = ncb
    ncb = _CACHED_NC[n_chunks]

    cs = n_chunks * CHUNK
    ws = n_chunks * M_KERN
    bw = BOOT_CH * M_KERN
    bc = BOOT_CH * CHUNK
    in_maps = []
    for i in range(N_CORES):
        phi_i = phi[:, i * cs : (i + 1) * cs]
        cw_i = cw[:, i * ws : (i + 1) * ws]
        in_maps.append(
            {
                "boot": np.ascontiguousarray(
                    np.concatenate([cw_i[:, :bw], phi_i[:, :bc]], axis=1)
                ),
                "phi": np.ascontiguousarray(phi_i),
                "cw": np.ascontiguousarray(cw_i),
            }
        )
    # The device occasionally throws a transient NRT_EXEC_UNIT_UNRECOVERABLE;
    # observed to clear after a short wait, so retry rather than fail.
    last_exc = None
    for attempt in range(3):
        if attempt:
            time.sleep(20)
        try:
            res = run_bass_kernel_spmd(
                ncb, in_maps, list(range(N_CORES)), trace=TRACE
            )
            break
        except (ImportError, TypeError, ValueError, AssertionError):
            raise
        except Exception as e:  # noqa: BLE001 - transient runtime faults
            last_exc = e
    else:
        raise last_exc
    LAST_RESULTS = res

    outT = np.concatenate(
        [res.results[i]["outT"] for i in range(N_CORES)], axis=1
    )  # (64, cap*CHUNK) uint8
    out = np.empty((n, M_KERN), dtype=np.float32)
    valid = slot_pid >= 0
    out[slot_pid[valid]] = outT[:, valid].T.astype(np.float32) * (1.0 / OSCALE)
    return np.ascontiguousarray(out)


# revision 49
# speedup vs baseline: 1.0514x; 1.0006x over previous
"""Kernel-correlation (gnn_message_passing) Trainium2 kernel.

out[i, m] = (1/128) * sum_{l<16} exp(-||normal[i] - kernel[m, l]||^2)

Strategy: out[:, m] is a fixed smooth function of the 3-D point normal[i]
(a Gauss transform of the 1024 kernel points).  Host-side, points are
bucketed into spatial boxes (side H) and the function is expanded per box
as a total-degree-D Taylor polynomial via Hermite recurrences (fast Gauss
transform).  With D=4 there are 35 monomial features, so each box's output
is one small GEMM: out[pts, 64] = phi[pts, 35] @ C[box][35, 64].  The
device kernel is a pure TensorEngine stream -- no exp, no reduction tree:
per 256-point chunk: LDWEIGHTS(C chunk) + MATMUL -> PSUM[64, 256], then a
PSUM->SBUF scaled-uint8 copy (alternating ScalarE/VectorE) and DMA out
(host dequantizes).  Weights are duplicated per chunk host-side so the
instruction stream is uniform and identical across the 8 SPMD cores; chunk
padding makes all shapes static.  DMA ring scheduling (boot DMA, piece
sizing, three-ring balancing) hides input latency behind the matmul
stream.  Total error ~1.0e-2 rel vs the 2e-2 gate.

Data-parallel over chunks on 8 NeuronCores, no collectives.
"""

import time

import numpy as np

N_TOTAL = 262144
N_CORES = 8
M_KERN = 64
K_SUB = 16
MK = M_KERN * K_SUB  # 1024

H = 0.55  # box side
D = 4  # Taylor total degree
CHUNK = 256  # points per matmul chunk
N_CHUNKS0 = 160  # chunks per core (seed-0 data needs ~158); grows if overflow
BOOT_CH = 12  # chunks whose weights+features ship in the single startup DMA
OSCALE = 2040.0  # uint8 output quantization: 255 / 0.125 (theoretical max out)

TRACE = False  # set by test.py to collect a neuron profile
LAST_RESULTS = None  # BassKernelResults of the most recent run

_CACHED_NC = {}  # n_chunks -> finalized Bacc

_EXPS = np.array(
    [
        (a, b, c)
        for a in range(D + 1)
        for b in range(D + 1 - a)
        for c in range(D + 1 - a - b)
    ],
    dtype=np.int64,
)
NF = len(_EXPS)  # 35


def _build_bass(n_chunks):
    import concourse.bacc as bacc
    import concourse.mybir as mybir
    from concourse.tile import TileContext

    f32 = mybir.dt.float32
    bf16 = mybir.dt.bfloat16
    u8 = mybir.dt.uint8
    MUL = mybir.AluOpType.mult

    nc = bacc.Bacc()
    # boot carries the first BOOT_CH chunks' weights AND features so one
    # startup DMA unblocks the first matmuls (everything else competes for
    # SDMA bandwidth behind it).
    boot = nc.declare_dram_parameter(
        "boot", [NF, BOOT_CH * (M_KERN + CHUNK)], bf16, isOutput=False
    )
    phi = nc.declare_dram_parameter(
        "phi", [NF, n_chunks * CHUNK], bf16, isOutput=False
    )
    cw = nc.declare_dram_parameter(
        "cw", [NF, n_chunks * M_KERN], bf16, isOutput=False
    )
    outT = nc.declare_dram_parameter(
        "outT", [M_KERN, n_chunks * CHUNK], u8, isOutput=True
    )

    # phi arrives in pieces alternating between the two HWDGE rings so the
    # first matmuls start while the bulk is in flight.  Sizes are chosen so
    # each ring's cumulative serial delivery (~70 GB/s/ring) stays ahead of
    # the matmul stream's consumption curve: small pieces early, growing.
    phi_pieces = [8, 8, 8, 16, 16]
    while sum(phi_pieces) < n_chunks - BOOT_CH:
        phi_pieces.append(min(24, n_chunks - BOOT_CH - sum(phi_pieces)))

    with TileContext(nc) as tc:
        with (
            tc.tile_pool(name="inp", bufs=1) as inp,
            tc.tile_pool(name="psump", bufs=8, space="PSUM") as psump,
            tc.tile_pool(name="stagep", bufs=4) as stagep,
        ):
            boott = inp.tile([NF, BOOT_CH * (M_KERN + CHUNK)], bf16, tag="boott")
            cwt = inp.tile([NF, (n_chunks - BOOT_CH) * M_KERN], bf16, tag="cwt")
            # One tile PER phi piece: a shared tile would add tile-granular
            # write-after-read hazards that serialize piece prefetch behind
            # the matmuls consuming earlier pieces.
            # boot goes first on the sync HWDGE ring; phi pieces alternate
            # sync/scalar HWDGE rings.  Outputs mostly ride the gpsimd SWDGE
            # ring so their descriptor generation never blocks input
            # prefetch.
            nc.sync.dma_start(out=boott[:], in_=boot[:])
            # early weights ride the scalar HWDGE ring (small, lands fast,
            # doesn't delay phi piece 1 behind a bulk transfer); the rest
            # rides SWDGE ahead of the out-DMAs
            cwe = 32 * M_KERN
            nc.scalar.dma_start(
                out=cwt[:, 0:cwe],
                in_=cw[:, BOOT_CH * M_KERN : BOOT_CH * M_KERN + cwe],
            )
            nc.gpsimd.dma_start(
                out=cwt[:, cwe:], in_=cw[:, BOOT_CH * M_KERN + cwe :]
            )
            phi_tiles = []  # (first_chunk, n_piece_chunks, tile)
            base = 0
            for pi, piece in enumerate(phi_pieces):
                sz = piece * CHUNK
                pt = inp.tile([NF, sz], bf16, tag=f"phi{pi}")
                src0 = BOOT_CH * CHUNK + base
                dq = nc.sync if pi % 2 == 0 else nc.scalar
                dq.dma_start(out=pt[:], in_=phi[:, src0 : src0 + sz])
                phi_tiles.append((BOOT_CH + base // CHUNK, piece, pt))
                base += sz

            # Two 256-pt matmuls land in one [64, 512] PSUM bank; each copy
            # instruction then moves 2 chunks (halves the per-instruction
            # read-write bubble), alternating ScalarE/VectorE.  GRP chunks
            # share one staging tile -> one out-DMA per GRP chunks.
            GRP = 16  # chunks per out-DMA
            assert n_chunks % GRP == 0
            ps = st = None
            piece_i = 0
            for c in range(n_chunks):
                if c < BOOT_CH:
                    lhsT = boott[:, c * M_KERN : (c + 1) * M_KERN]
                    rb = BOOT_CH * M_KERN + c * CHUNK
                    rhs = boott[:, rb : rb + CHUNK]
                else:
                    cc = c - BOOT_CH
                    lhsT = cwt[:, cc * M_KERN : (cc + 1) * M_KERN]
                    while (
                        c >= phi_tiles[piece_i][0] + phi_tiles[piece_i][1]
                    ):
                        piece_i += 1
                    pc0, _, pt = phi_tiles[piece_i]
                    off = (c - pc0) * CHUNK
                    rhs = pt[:, off : off + CHUNK]
                if c % 2 == 0:
                    ps = psump.tile([M_KERN, 2 * CHUNK], f32, tag="ps")
                nc.tensor.matmul(
                    out=ps[:, (c % 2) * CHUNK : (c % 2 + 1) * CHUNK],
                    lhsT=lhsT,
                    rhs=rhs,
                    start=True,
                    stop=True,
                )
                if c % GRP == 0:
                    st = stagep.tile([M_KERN, GRP * CHUNK], u8, tag="st")
                if c % 2 == 1:
                    sl = st[:, (c % GRP - 1) * CHUNK : (c % GRP + 1) * CHUNK]
                    # PSUM -> SBUF scaled uint8 quantized copy
                    if c % 4 == 1:
                        nc.scalar.mul(out=sl, in_=ps[:], mul=OSCALE)
                    else:
                        nc.vector.tensor_scalar(
                            out=sl, in0=ps[:], scalar1=OSCALE, scalar2=None,
                            op0=MUL,
                        )
                if c // GRP == n_chunks // GRP - 1:
                    # tail taper: the final group leaves as quarter-DMAs as
                    # soon as each 4-chunk sub-block's copies complete, on
                    # the by-then-idle HWDGE rings
                    if c % 4 == 3:
                        q0 = (c % GRP - 3) * CHUNK
                        g0c = (c - 3) * CHUNK
                        dq = nc.sync if (c % GRP) // 4 % 2 == 0 else nc.scalar
                        dq.dma_start(
                            out=outT[:, g0c : g0c + 4 * CHUNK],
                            in_=st[:, q0 : q0 + 4 * CHUNK],
                        )
                elif c % GRP == GRP - 1:
                    g0 = (c - GRP + 1) * CHUNK
                    if c >= n_chunks - 2 * GRP:
                        # second-to-last group: two half-DMAs on the
                        # by-then-idle HWDGE rings
                        hw = GRP * CHUNK // 2
                        nc.sync.dma_start(
                            out=outT[:, g0 : g0 + hw], in_=st[:, 0:hw]
                        )
                        nc.scalar.dma_start(
                            out=outT[:, g0 + hw : g0 + 2 * hw],
                            in_=st[:, hw : 2 * hw],
                        )
                    else:
                        # ring balancing: the HWDGE rings carry the phi
                        # pieces (~1.4 MB each), so most outs ride SWDGE;
                        # a couple go to sync once its pieces thin out
                        dq = nc.sync if (c // GRP) >= 6 else nc.gpsimd
                        dq.dma_start(
                            out=outT[:, g0 : g0 + GRP * CHUNK], in_=st[:]
                        )
    return nc


def _hermite_g(t, D):
    """g_p(t) = H_p(t) e^{-t^2} / p!  for p = 0..D (physicists' Hermite)."""
    e = np.exp(-(t**2))
    H_ = np.empty((D + 1,) + t.shape)
    H_[0] = 1.0
    if D >= 1:
        H_[1] = 2 * t
    for p in range(2, D + 1):
        H_[p] = 2 * t * H_[p - 1] - 2 * (p - 1) * H_[p - 2]
    fact = np.cumprod(np.concatenate([[1.0], np.arange(1.0, D + 1)]))
    return H_ * e / fact.reshape((D + 1,) + (1,) * t.ndim)


def _prep(normal, kern):
    """Box the points, build per-box Taylor coefficients and per-point
    monomial features, lay both out as fixed-size per-chunk arrays."""
    import ml_dtypes

    bf = ml_dtypes.bfloat16
    x = np.asarray(normal, dtype=np.float64)
    kf = np.asarray(kern, dtype=np.float64).reshape(MK, 3)
    n = x.shape[0]

    L = np.abs(x).max() + 1e-6
    idx3 = np.floor((x + L) / H).astype(np.int64)
    nside = int(np.ceil(2 * L / H))
    bid = (idx3[:, 0] * nside + idx3[:, 1]) * nside + idx3[:, 2]
    uniq, inv = np.unique(bid, return_inverse=True)
    nbox = len(uniq)
    iz = uniq % nside
    iy = (uniq // nside) % nside
    ix = uniq // (nside * nside)
    centers = np.stack([ix, iy, iz], 1) * H - L + H / 2  # (nbox, 3)

    # per-box Taylor coefficients about the box center (Hermite recurrence),
    # summed over each m's 16 kernel points; includes the 1/128 out-scale
    t = kf[None, :, :] - centers[:, None, :]  # (nbox, 1024, 3)
    g = _hermite_g(t, D)  # (D+1, nbox, 1024, 3)
    prod = g[_EXPS[:, 0], :, :, 0] * g[_EXPS[:, 1], :, :, 1] * g[_EXPS[:, 2], :, :, 2]
    C = np.transpose(
        prod.reshape(NF, nbox, M_KERN, K_SUB).sum(-1), (1, 0, 2)
    )  # (nbox, NF, 64)
    C = np.ascontiguousarray(C / 128.0)

    # per-point monomial features of (x - center(box))
    delta = x - centers[inv]
    powd = [np.vander(delta[:, d], D + 1, increasing=True) for d in range(3)]
    feats = (
        powd[0][:, _EXPS[:, 0]] * powd[1][:, _EXPS[:, 1]] * powd[2][:, _EXPS[:, 2]]
    )  # (n, NF)

    # chunk layout: points sorted by box, each box padded to CHUNK multiple
    order = np.argsort(inv, kind="stable")
    cnt = np.bincount(inv, minlength=nbox)
    box_chunks = -(-cnt // CHUNK)  # ceil
    total_chunks = int(box_chunks.sum())
    n_chunks = N_CHUNKS0
    while n_chunks * N_CORES < total_chunks:
        n_chunks += 32
    cap = n_chunks * N_CORES

    chunk_box = np.full(cap, -1, dtype=np.int64)
    slot_pid = np.full(cap * CHUNK, -1, dtype=np.int64)
    chunk_starts = np.concatenate([[0], np.cumsum(box_chunks)])[:-1]
    pt_starts = np.concatenate([[0], np.cumsum(cnt)])[:-1]
    # vectorized scatter of point-ids into padded slots
    seq = np.arange(n)
    box_of_pt = inv[order]
    rank_in_box = seq - pt_starts[box_of_pt]
    slot = (
        chunk_starts[box_of_pt] * CHUNK
        + (rank_in_box // CHUNK) * CHUNK
        + rank_in_box % CHUNK
    )
    slot_pid[slot] = order
    for b_ids, c_starts, c_counts in [(np.arange(nbox), chunk_starts, box_chunks)]:
        reps = np.repeat(b_ids, c_counts)
        chunk_box[: len(reps)] = reps

    # phi: (NF, cap*CHUNK) bf16, zero on padding
    phi = np.zeros((NF, cap * CHUNK), dtype=bf)
    valid = slot_pid >= 0
    phi[:, valid] = feats[slot_pid[valid]].astype(bf).T

    # cw: (NF, cap*64) bf16, per-chunk duplicated box coefficients
    cw = np.zeros((NF, cap, M_KERN), dtype=bf)
    vc = chunk_box >= 0
    cw[:, vc, :] = C[chunk_box[vc]].astype(bf).transpose(1, 0, 2)
    cw = cw.reshape(NF, cap * M_KERN)

    return phi, cw, slot_pid, n_chunks


def kernel(normal, neighbour, kernel):  # noqa: A002 - harness-fixed names
    global LAST_RESULTS
    from concourse.bass_utils import run_bass_kernel_spmd

    n = np.asarray(normal).shape[0]
    phi, cw, slot_pid, n_chunks = _prep(normal, kernel)

    if n_chunks not in _CACHED_NC:
        ncb = _build_bass(n_chunks)
        if not ncb.is_finalized():
            ncb.finalize()
        _CACHED_NC[n_chunks] = ncb
    ncb = _CACHED_NC[n_chunks]

    cs = n_chunks * CHUNK
    ws = n_chunks * M_KERN
    bw = BOOT_CH * M_KERN
    bc = BOOT_CH * CHUNK
    in_maps = []
    for i in range(N_CORES):
        phi_i = phi[:, i * cs : (i + 1) * cs]
        cw_i = cw[:, i * ws : (i + 1) * ws]
        in_maps.append(
            {
                "boot": np.ascontiguousarray(
                    np.concatenate([cw_i[:, :bw], phi_i[:, :bc]], axis=1)
                ),
                "phi": np.ascontiguousarray(phi_i),
                "cw": np.ascontiguousarray(cw_i),
            }
        )
    # The device occasionally throws a transient NRT_EXEC_UNIT_UNRECOVERABLE;
    # observed to clear after a short wait, so retry rather than fail.
    last_exc = None
    for attempt in range(3):
        if attempt:
            time.sleep(20)
        try:
            res = run_bass_kernel_spmd(
                ncb, in_maps, list(range(N_CORES)), trace=TRACE
            )
            break
        except (ImportError, TypeError, ValueError, AssertionError):
            raise
        except Exception as e:  # noqa: BLE001 - transient runtime faults
            last_exc = e
    else:
        raise last_exc
    LAST_RESULTS = res

    outT = np.concatenate(
        [res.results[i]["outT"] for i in range(N_CORES)], axis=1
    )  # (64, cap*CHUNK) uint8
    out = np.empty((n, M_KERN), dtype=np.float32)
    valid = slot_pid >= 0
    out[slot_pid[valid]] = outT[:, valid].T.astype(np.float32) * (1.0 / OSCALE)
    return np.ascontiguousarray(out)
